# revision 1
# baseline (speedup 1.0000x reference)
"""Self-contained Trainium2 Bass kernel for nn_MultiHeadAttention_71528385347884.

Strategy: head tensor-parallel across 8 cores (2 heads/core). Per core:
  - QKV projection with x transposed (feature-major q/k, token-major v)
  - RoPE via host-side A/B weight-column packing (no cross-partition ops)
  - causal attention in [s,t] score layout, softmax without max-subtraction
    (scores are bounded ~|4.5|), denominator via all-ones matmul
  - output projection exploits the reference's scrambled
    transpose(0,2,1,3).reshape(B,T,C): each core produces disjoint output
    rows -> host gather is pure concatenation.
"""

import math
import numpy as np
import ml_dtypes

# ---- problem constants (hardcoded; kernel.py must not read spec/reference) ----
B = 2
T = 2048          # sequence length per batch
C = 2048          # model dim
Dh = 128          # head dim
N_HEAD = 16
N_CORES = 8
H_LOCAL = 2       # heads per core
ROPE_BASE = 10000.0
SCALE = 1.0 / math.sqrt(Dh)

BF16 = ml_dtypes.bfloat16


class Cfg:
    """Size parameters so the same builder runs a small CoreSim config."""

    def __init__(self, B=B, T=T, C=C):
        assert T % 512 == 0 and C % 128 == 0
        self.B = B
        self.T = T
        self.C = C
        self.NCC = C // 128        # contraction chunks for qkv matmuls
        self.BT = B * T
        self.NT = T // 512         # 512-wide t-tiles per batch
        self.GRP = C // Dh         # tokens folded per output row by the reshape
        self.TAU = T // self.GRP   # output rows per (b, h); must be 128
        assert self.TAU == 128
        self.ET = max(1, C // 512)  # 512-wide e-tiles of the output
        self.JQK = 4 * 128         # qA,qB,kA,kB feature blocks
        self.JV = H_LOCAL * 128


FULL = Cfg()


# =====================================================================
# Device program builder
# =====================================================================

def build_nc(cfg: Cfg, debug=False):
    import concourse.bass as bass
    import concourse.mybir as mybir
    import concourse.tile as tile
    from concourse import bacc

    f32 = mybir.dt.float32
    bf16 = mybir.dt.bfloat16
    Exp = mybir.ActivationFunctionType.Exp
    Copy = mybir.ActivationFunctionType.Copy

    nc = bacc.Bacc(None, target_bir_lowering=False, debug=debug)

    xt_d = nc.dram_tensor("xt", [128, cfg.NCC, cfg.BT], bf16, kind="ExternalInput")
    wqk_d = nc.dram_tensor("wqk", [128, cfg.NCC, cfg.JQK], bf16, kind="ExternalInput")
    wv_d = nc.dram_tensor("wv", [128, cfg.NCC, cfg.JV], bf16, kind="ExternalInput")
    wp_d = nc.dram_tensor("wp", [128, cfg.GRP, cfg.C], bf16, kind="ExternalInput")
    cc2_d = nc.dram_tensor("cc2", [128, cfg.T], f32, kind="ExternalInput")
    spm_d = nc.dram_tensor("spm", [128, cfg.T], f32, kind="ExternalInput")
    smp_d = nc.dram_tensor("smp", [128, cfg.T], f32, kind="ExternalInput")
    masks_d = nc.dram_tensor("masks", [128, 4, 512], bf16, kind="ExternalInput")
    out_d = nc.dram_tensor("out", [cfg.B, H_LOCAL, 128, cfg.C], f32,
                           kind="ExternalOutput")

    with tile.TileContext(nc) as tc:
        with tc.tile_pool(name="persist", bufs=1) as persist:
            # ---- persistent SBUF state ----
            wqk_sb = persist.tile([128, cfg.NCC, cfg.JQK], bf16, name="wqk_sb",
                                  tag="wqk_sb")
            wv_sb = persist.tile([128, cfg.NCC, cfg.JV], bf16, name="wv_sb",
                                 tag="wv_sb")
            cc2_sb = persist.tile([128, cfg.T], f32, name="cc2_sb", tag="cc2_sb")
            spm_sb = persist.tile([128, cfg.T], f32, name="spm_sb", tag="spm_sb")
            smp_sb = persist.tile([128, cfg.T], f32, name="smp_sb", tag="smp_sb")
            masks_sb = persist.tile([128, 4, 512], bf16, name="masks_sb",
                                    tag="masks_sb")
            ones_sb = persist.tile([128, 128], bf16, name="ones_sb", tag="ones_sb")

            nc.sync.dma_start(wqk_sb[:], wqk_d[:])
            nc.sync.dma_start(wv_sb[:], wv_d[:])
            nc.sync.dma_start(cc2_sb[:], cc2_d[:])
            nc.sync.dma_start(spm_sb[:], spm_d[:])
            nc.sync.dma_start(smp_sb[:], smp_d[:])
            nc.sync.dma_start(masks_sb[:], masks_d[:])
            nc.vector.memset(ones_sb[:], 1.0)

            # per-(b, head-or-tile) persistent tensors
            qA_sb, qB_sb, kA_sb, kB_sb = {}, {}, {}, {}
            v_sb, attn_sb = {}, {}
            for b in range(cfg.B):
                for nm, d in (("qA", qA_sb), ("qB", qB_sb),
                              ("kA", kA_sb), ("kB", kB_sb)):
                    d[b] = persist.tile([128, cfg.T], bf16, name=f"{nm}_{b}",
                                        tag=f"{nm}_{b}")
                for hl in range(H_LOCAL):
                    v_sb[(b, hl)] = persist.tile(
                        [128, cfg.T // 128, 128], bf16,
                        name=f"v_{b}_{hl}", tag=f"v_{b}_{hl}")
                    attn_sb[(b, hl)] = persist.tile(
                        [128, cfg.T], bf16,
                        name=f"at_{b}_{hl}", tag=f"at_{b}_{hl}")

            # ================= Phase A: V projection (token-major) ========
            with (
                tc.tile_pool(name="xa_pool", bufs=6) as xa_pool,
                tc.tile_pool(name="vps", bufs=8, space="PSUM") as vps,
            ):
                for b in range(cfg.B):
                    for tt in range(cfg.NT):
                        bt0 = b * cfg.T + tt * 512
                        pv = [vps.tile([128, cfg.JV], f32, name=f"pv_{b}_{tt}_{i}",
                                       tag="pv") for i in range(4)]
                        for ccs in range(cfg.NCC):
                            xa = xa_pool.tile([128, 512], bf16,
                                              name=f"xa_{b}_{tt}_{ccs}", tag="xa")
                            nc.sync.dma_start(xa[:], xt_d[:, ccs, bt0:bt0 + 512])
                            for s4 in range(4):
                                nc.tensor.matmul(
                                    pv[s4][:], xa[:, s4 * 128:(s4 + 1) * 128],
                                    wv_sb[:, ccs, :],
                                    start=(ccs == 0), stop=(ccs == cfg.NCC - 1))
                        for s4 in range(4):
                            sc = tt * 4 + s4
                            for hl in range(H_LOCAL):
                                nc.scalar.activation(
                                    v_sb[(b, hl)][:, sc, :],
                                    pv[s4][:, hl * 128:(hl + 1) * 128], Copy)

            # ================= Phase B: Q/K projection + RoPE =============
            with (
                tc.tile_pool(name="xb_pool", bufs=6) as xb_pool,
                tc.tile_pool(name="rtmp", bufs=4) as rtmp,
                tc.tile_pool(name="qkps", bufs=8, space="PSUM") as qkps,
            ):
                for b in range(cfg.B):
                    for tt in range(cfg.NT):
                        bt0 = b * cfg.T + tt * 512
                        tl = slice(tt * 512, (tt + 1) * 512)  # within-batch cols
                        pj = [qkps.tile([128, 512], f32, name=f"pj_{b}_{tt}_{j}",
                                        tag="pj") for j in range(4)]
                        for ccs in range(cfg.NCC):
                            xb = xb_pool.tile([128, 512], bf16,
                                              name=f"xb_{b}_{tt}_{ccs}", tag="xb")
                            nc.sync.dma_start(xb[:], xt_d[:, ccs, bt0:bt0 + 512])
                            for jc in range(4):
                                nc.tensor.matmul(
                                    pj[jc][:],
                                    wqk_sb[:, ccs, jc * 128:(jc + 1) * 128],
                                    xb[:],
                                    start=(ccs == 0), stop=(ccs == cfg.NCC - 1))
                        # rope: rotA = A*C2 + B*S+-,  rotB = B*C2 + A*S-+
                        for (Aps, Bps, dA, dB) in (
                            (pj[0], pj[1], qA_sb[b], qB_sb[b]),
                            (pj[2], pj[3], kA_sb[b], kB_sb[b]),
                        ):
                            m1 = rtmp.tile([128, 512], f32, name="m1", tag="rt")
                            m2 = rtmp.tile([128, 512], f32, name="m2", tag="rt")
                            nc.vector.tensor_mul(m1[:], Aps[:], cc2_sb[:, tl])
                            nc.vector.tensor_mul(m2[:], Bps[:], spm_sb[:, tl])
                            nc.vector.tensor_add(dA[:, tl], m1[:], m2[:])
                            m3 = rtmp.tile([128, 512], f32, name="m3", tag="rt")
                            m4 = rtmp.tile([128, 512], f32, name="m4", tag="rt")
                            nc.vector.tensor_mul(m3[:], Bps[:], cc2_sb[:, tl])
                            nc.vector.tensor_mul(m4[:], Aps[:], smp_sb[:, tl])
                            nc.vector.tensor_add(dB[:, tl], m3[:], m4[:])

            # ================= Phase C: causal attention ==================
            with (
                tc.tile_pool(name="probs_pool", bufs=6) as probs_pool,
                tc.tile_pool(name="rec_pool", bufs=2) as rec_pool,
                tc.tile_pool(name="sps", bufs=4, space="PSUM") as sps,
                tc.tile_pool(name="ops", bufs=2, space="PSUM") as ops,
                tc.tile_pool(name="dps", bufs=2, space="PSUM") as dps,
            ):
                for b in range(cfg.B):
                    qA, qB = qA_sb[b], qB_sb[b]
                    kA, kB = kA_sb[b], kB_sb[b]
                    for tt in range(cfg.NT):
                        tl = slice(tt * 512, (tt + 1) * 512)
                        n_sc = (tt + 1) * 4
                        po = [ops.tile([128, 512], f32, name=f"po_{b}_{tt}_{h}",
                                       tag="po") for h in range(2)]
                        pd = [dps.tile([128, 512], f32, name=f"pd_{b}_{tt}_{h}",
                                       tag="pd") for h in range(2)]
                        for sc in range(n_sc):
                            sl = slice(sc * 128, (sc + 1) * 128)
                            ph = [sps.tile([128, 512], f32,
                                           name=f"ps_{b}_{tt}_{sc}_{h}", tag="ps")
                                  for h in range(2)]
                            # head 0 lives on partitions 0:64 of A/B tiles
                            nc.tensor.matmul(ph[0][:], kA[0:64, sl], qA[0:64, tl],
                                             start=True, stop=False)
                            nc.tensor.matmul(ph[0][:], kB[0:64, sl], qB[0:64, tl],
                                             start=False, stop=True)
                            # head 1 on partitions 64:128
                            nc.tensor.matmul(ph[1][:], kA[64:128, sl],
                                             qA[64:128, tl], start=True, stop=False)
                            nc.tensor.matmul(ph[1][:], kB[64:128, sl],
                                             qB[64:128, tl], start=False, stop=True)
                            for h in range(2):
                                pr = probs_pool.tile([128, 512], bf16,
                                                     name=f"pr_{h}", tag="pr")
                                nc.scalar.activation(pr[:], ph[h][:], Exp,
                                                     scale=SCALE)
                                if sc >= tt * 4:  # diagonal block: causal mask
                                    nc.vector.tensor_mul(
                                        pr[:], pr[:], masks_sb[:, sc - tt * 4, :])
                                nc.tensor.matmul(po[h][:], v_sb[(b, h)][:, sc, :],
                                                 pr[:], start=(sc == 0),
                                                 stop=(sc == n_sc - 1))
                                nc.tensor.matmul(pd[h][:], ones_sb[:], pr[:],
                                                 start=(sc == 0),
                                                 stop=(sc == n_sc - 1))
                        for h in range(2):
                            rec = rec_pool.tile([128, 512], f32, name=f"rec_{h}",
                                                tag="rec")
                            nc.vector.reciprocal(rec[:], pd[h][:])
                            nc.vector.tensor_mul(attn_sb[(b, h)][:, tl],
                                                 po[h][:], rec[:])

            # ================= Phase D: output projection =================
            with (
                tc.tile_pool(name="wpe_pool", bufs=2) as wpe_pool,
                tc.tile_pool(name="ostg_pool", bufs=4) as ostg_pool,
                tc.tile_pool(name="pps", bufs=4, space="PSUM") as pps,
            ):
                for et in range(cfg.ET):
                    el = slice(et * 512, (et + 1) * 512)
                    ew = min(512, cfg.C)
                    wpe = wpe_pool.tile([128, cfg.GRP, ew], bf16,
                                        name=f"wpe_{et}", tag="wpe")
                    nc.sync.dma_start(wpe[:], wp_d[:, :, el])
                    for b in range(cfg.B):
                        for hl in range(H_LOCAL):
                            pp = pps.tile([128, ew], f32,
                                          name=f"pp_{et}_{b}_{hl}", tag="pp")
                            at = attn_sb[(b, hl)]
                            for u in range(cfg.GRP):
                                nc.tensor.matmul(pp[:], at[:, u::cfg.GRP],
                                                 wpe[:, u, :],
                                                 start=(u == 0),
                                                 stop=(u == cfg.GRP - 1))
                            stg = ostg_pool.tile([128, ew], f32,
                                                 name=f"stg_{et}_{b}_{hl}",
                                                 tag="stg")
                            nc.scalar.activation(stg[:], pp[:], Copy)
                            nc.sync.dma_start(out_d[b, hl, :, el], stg[:])

    nc.compile()
    return nc


# =====================================================================
# Host-side input prep / output gather
# =====================================================================

def _part_major(a2d, ncc):
    """[ncc*128, F] -> [128, ncc, F] with row r = chunk*128 + p."""
    F = a2d.shape[1]
    return np.ascontiguousarray(
        a2d.reshape(ncc, 128, F).transpose(1, 0, 2))


def make_trig(cfg: Cfg):
    pos = np.arange(cfg.T, dtype=np.float64)[None, :]        # [1,T]
    j = np.arange(64, dtype=np.float64)[:, None]             # [64,1]
    inv = ROPE_BASE ** (-2.0 * j / Dh)
    ang = pos * inv                                          # [64,T]
    sin = np.sin(ang).astype(np.float32)
    cos = np.cos(ang).astype(np.float32)
    cc2 = np.concatenate([cos, cos], axis=0)                 # [128,T]
    spm = np.concatenate([-sin, sin], axis=0)
    smp = np.concatenate([sin, -sin], axis=0)
    return cc2, spm, smp


def make_masks():
    p = np.arange(128)[:, None]
    jj = np.arange(512)[None, :]
    masks = np.stack([( (m * 128 + p) <= jj ) for m in range(4)], axis=1)
    return masks.astype(BF16)                                # [128,4,512]


def make_in_maps(x, w_qkv, w_proj, cfg: Cfg = FULL, n_cores=N_CORES,
                 n_head=N_HEAD):
    x = np.asarray(x, np.float32)
    w_qkv = np.asarray(w_qkv, np.float32)
    w_proj = np.asarray(w_proj, np.float32)
    Cm = cfg.C

    xT = np.ascontiguousarray(x.reshape(cfg.BT, Cm).T)       # [C, BT]
    xt = _part_major(xT, cfg.NCC).astype(BF16)
    wp = _part_major(w_proj, cfg.GRP).astype(BF16)
    cc2, spm, smp = make_trig(cfg)
    masks = make_masks()

    wq = w_qkv[:, 0:Cm]
    wk = w_qkv[:, Cm:2 * Cm]
    wv_all = w_qkv[:, 2 * Cm:3 * Cm]

    in_maps = []
    for c in range(n_cores):
        h0, h1 = 2 * c, 2 * c + 1
        q0 = wq[:, h0 * 128:(h0 + 1) * 128]
        q1 = wq[:, h1 * 128:(h1 + 1) * 128]
        k0 = wk[:, h0 * 128:(h0 + 1) * 128]
        k1 = wk[:, h1 * 128:(h1 + 1) * 128]
        qA = np.concatenate([q0[:, 0:64], q1[:, 64:128]], axis=1)
        qB = np.concatenate([q0[:, 64:128], q1[:, 0:64]], axis=1)
        kA = np.concatenate([k0[:, 0:64], k1[:, 64:128]], axis=1)
        kB = np.concatenate([k0[:, 64:128], k1[:, 0:64]], axis=1)
        wqk = _part_major(
            np.concatenate([qA, qB, kA, kB], axis=1), cfg.NCC).astype(BF16)
        wv = _part_major(
            np.concatenate([wv_all[:, h0 * 128:(h0 + 1) * 128],
                            wv_all[:, h1 * 128:(h1 + 1) * 128]], axis=1),
            cfg.NCC).astype(BF16)
        in_maps.append(dict(xt=xt, wqk=wqk, wv=wv, wp=wp,
                            cc2=cc2, spm=spm, smp=smp, masks=masks))
    return in_maps


def gather(outs, cfg: Cfg = FULL):
    """outs: per-core [B, H_LOCAL, 128, C] -> full [B, T, C]."""
    rows = np.concatenate(
        [o.reshape(cfg.B, H_LOCAL * 128, cfg.C) for o in outs], axis=1)
    return np.ascontiguousarray(rows.reshape(cfg.B, cfg.T, cfg.C))


# =====================================================================
# Public entry point
# =====================================================================

_NC_CACHE = {}


def get_nc(debug=False):
    key = ("full", debug)
    if key not in _NC_CACHE:
        _NC_CACHE[key] = build_nc(FULL, debug=debug)
    return _NC_CACHE[key]


def kernel(x, w_qkv, w_proj):
    from concourse.bass_utils import run_bass_kernel_spmd
    nc = get_nc()
    in_maps = make_in_maps(x, w_qkv, w_proj)
    res = run_bass_kernel_spmd(nc, in_maps, list(range(N_CORES)))
    return gather([res.results[c]["out"] for c in range(N_CORES)])
